# revision 3
# baseline (speedup 1.0000x reference)
"""MLA (multi-head latent) causal attention on 8 Trainium2 NeuronCores. v2.

Sharding: batch(4) x head-group(2) mesh over 8 cores. Core c handles batch
c//2 and heads [8*(c%2), 8*(c%2)+8). Latent KV projections are recomputed per
head-group (MLA: latent shared across heads). Each core produces a partial
output (its head-group's contribution to y @ wo^T); the host sums the two
partials per batch.

v2 design (all matmul operands bf16, fp32 PSUM accumulation; ~0.5% rel rms):
  - fused schedule per 512-token chunk n: A(n) projections -> B(n) attention
    over query chunk n -> C(n) output projection, with A(n+1) slices and
    C(n-1) groups interleaved into B(n) to keep the PE dense while ACT does
    the exps.
  - q and y never round-trip DRAM: q_sb/y_sb are double-buffered SBUF tiles.
  - softmax row-sum: per-key-block exps are accumulated into esum on the DVE
    (sliced adds), one small ones-matmul contracts the final 128 partitions.
  - causal mask: Pool affine_select zeroes the upper triangle of the diagonal
    128x128 sub-block of exp (no additive -1e9 pass on the scores).
  - weights (wq, wo, wkv, wku, wvu) and rope tables resident in SBUF, bf16.
"""

import math
from collections import deque
from contextlib import ExitStack

import numpy as np
import ml_dtypes

import concourse.bass as bass
import concourse.mybir as mybir
import concourse.tile as tile
from concourse import bacc
from concourse.bass_utils import run_bass_kernel_spmd

# Problem shape (hardcoded per contract).
B, T, C = 4, 2048, 2048
H, D, L = 16, 128, 512
HG = 8           # heads per core
N_CORES = 8
P = 128
KC = C // P      # 16 contraction chunks over C
LC = L // P      # 4 chunks over L
NQ = T // 512    # 4 query chunks of 512
NT = T // P      # 16 key chunks of 128
SCALE = 1.0 / math.sqrt(D)

F32 = mybir.dt.float32
BF16 = mybir.dt.bfloat16

_cached = {}


def _build_program():
    nc = bacc.Bacc()

    xT = nc.dram_tensor("xT", [C, T], BF16, kind="ExternalInput").ap()
    wqT = nc.dram_tensor("wqT", [C, HG * D], BF16, kind="ExternalInput").ap()
    wkvT = nc.dram_tensor("wkvT", [C, L], BF16, kind="ExternalInput").ap()
    wkuT = nc.dram_tensor("wkuT", [L, D], BF16, kind="ExternalInput").ap()
    wvuT = nc.dram_tensor("wvuT", [L, D], BF16, kind="ExternalInput").ap()
    woT = nc.dram_tensor("woT", [HG * D, C], BF16, kind="ExternalInput").ap()
    c2 = nc.dram_tensor("c2", [P, T], BF16, kind="ExternalInput").ap()
    s2 = nc.dram_tensor("s2", [P, T], BF16, kind="ExternalInput").ap()
    outp = nc.dram_tensor("outp", [T, C], BF16, kind="ExternalOutput").ap()

    xT_r = xT.rearrange("(kc p) t -> p kc t", p=P)
    wqT_r = wqT.rearrange("(kc p) e -> p kc e", p=P)
    wkvT_r = wkvT.rearrange("(kc p) l -> p kc l", p=P)
    wkuT_r = wkuT.rearrange("(lc p) d -> p lc d", p=P)
    wvuT_r = wvuT.rearrange("(lc p) d -> p lc d", p=P)
    woT_r = woT.rearrange("(h p) c -> p h c", p=P)

    with tile.TileContext(nc) as tc, ExitStack() as top:
        persist = top.enter_context(tc.tile_pool(name="persist", bufs=1))
        pall = top.enter_context(tc.tile_pool(name="pall", bufs=8, space="PSUM"))
        xpool = top.enter_context(tc.tile_pool(name="xpool", bufs=2))
        qpool = top.enter_context(tc.tile_pool(name="qpool", bufs=2))
        ypool = top.enter_context(tc.tile_pool(name="ypool", bufs=2))
        bpool = top.enter_context(tc.tile_pool(name="bpool", bufs=2))
        rpool = top.enter_context(tc.tile_pool(name="rpool", bufs=2))
        ropool = top.enter_context(tc.tile_pool(name="ropool", bufs=1))
        kvpool = top.enter_context(tc.tile_pool(name="kvpool", bufs=1))

        # ---- resident tensors ----
        wkv_sb = persist.tile([P, KC, L], BF16)          # 16KB/part
        x0 = xpool.tile([P, KC, 512], BF16, tag="xn", name="xn0")
        # interleave wkv and x(0) per-kc slices so chunk-0 kv matmuls can
        # start as soon as the first slices land
        for kc in range(KC):
            nc.sync.dma_start(wkv_sb[:, kc, :], wkvT_r[:, kc, :])
            nc.sync.dma_start(x0[:, kc, :], xT_r[:, kc, bass.ts(0, 512)])

        wku_sb = persist.tile([P, LC, D], BF16)
        wvu_sb = persist.tile([P, LC, D], BF16)
        ones = persist.tile([P, P], BF16)
        nc.sync.dma_start(wku_sb[:], wkuT_r)
        nc.sync.dma_start(wvu_sb[:], wvuT_r)
        nc.vector.memset(ones[:], 1.0)

        cspool = top.enter_context(tc.tile_pool(name="cspool", bufs=2))

        def fetch_cs(n):
            c2n = cspool.tile([P, 512], BF16, tag="c2n", name=f"c2n{n}")
            s2n = cspool.tile([P, 512], BF16, tag="s2n", name=f"s2n{n}")
            nc.sync.dma_start(c2n[:], c2[:, bass.ts(n, 512)])
            nc.sync.dma_start(s2n[:], s2[:, bass.ts(n, 512)])
            return c2n, s2n

        csns = [fetch_cs(0), None, None, None]

        # wq in 4 head-pair column slices (first q head needs only slice 0)
        wq_sb = persist.tile([P, KC, HG * P], BF16)      # 32KB/part
        for g in range(4):
            nc.sync.dma_start(wq_sb[:, :, bass.ts(g, 2 * P)],
                              wqT_r[:, :, bass.ts(g, 2 * P)])
        wo_sb = persist.tile([P, HG, C], BF16)           # 32KB/part
        nc.sync.dma_start(wo_sb[:], woT_r)

        # per-chunk k/v slabs (separate tiles so B(n) reads don't serialize
        # against A(n+1) writes)
        k_slabs = [persist.tile([P, 4, P], BF16, name=f"kslab{i}")
                   for i in range(NQ)]
        v_slabs = [persist.tile([P, 4, P], BF16, name=f"vslab{i}")
                   for i in range(NQ)]

        q_sbs = [qpool.tile([P, HG, 512], BF16, tag="qsb", name=f"qsb{i % 2}")
                 for i in range(2)]
        y_sbs = [ypool.tile([P, HG, 512], BF16, tag="ysb", name=f"ysb{i % 2}")
                 for i in range(2)]

        def rope(dst, ps, n):
            # dst = qq * c2 + swap64(qq) * s2  (per 512-token chunk n)
            c2n, s2n = csns[n]
            qq = ropool.tile([P, 512], BF16, tag="qq")
            qs = ropool.tile([P, 512], BF16, tag="qs")
            m1 = ropool.tile([P, 512], BF16, tag="m1")
            nc.scalar.copy(qq[:], ps[:])
            nc.vector.tensor_copy(qs[0:64, :], qq[64:128, :])
            nc.vector.tensor_copy(qs[64:128, :], qq[0:64, :])
            nc.vector.tensor_tensor(m1[:], qq[:], c2n[:],
                                    mybir.AluOpType.mult)
            nc.vector.tensor_tensor(qs[:], qs[:], s2n[:],
                                    mybir.AluOpType.mult)
            nc.vector.tensor_tensor(dst, m1[:], qs[:], mybir.AluOpType.add)

        # ---------- phase A slice emitters ----------
        def emit_kv(n, xn):
            # latent kv for chunk n (kc-outer: 4 psum banks), then k-up+rope
            # and v in [t, d] layout
            kvps = [pall.tile([P, 512], F32, tag="pa", name=f"kvps{n}_{i}")
                    for i in range(LC)]
            for kc in range(KC):
                for lc in range(LC):
                    nc.tensor.matmul(kvps[lc][:],
                                     wkv_sb[:, kc, bass.ts(lc, P)],
                                     xn[:, kc, :],
                                     start=(kc == 0), stop=(kc == KC - 1))
            kvn = kvpool.tile([P, LC, 512], BF16, tag="kvn")
            for lc in range(LC):
                nc.scalar.copy(kvn[:, lc, :], kvps[lc][:])

            kp = pall.tile([P, 512], F32, tag="pa", name=f"kp{n}")
            for lc in range(LC):
                nc.tensor.matmul(kp[:], wku_sb[:, lc, :], kvn[:, lc, :],
                                 start=(lc == 0), stop=(lc == LC - 1))
            kdst = k_slabs[n][:].rearrange("p a b -> p (a b)")
            rope(kdst, kp, n)

            vps = pall.tile([P, 4, P], F32, tag="pa", name=f"vps{n}")
            for i in range(4):
                for lc in range(LC):
                    nc.tensor.matmul(vps[:, i, :],
                                     kvn[:, lc, bass.ts(i, P)],
                                     wvu_sb[:, lc, :],
                                     start=(lc == 0), stop=(lc == LC - 1))
            for i in range(4):
                nc.scalar.copy(v_slabs[n][:, i, :], vps[:, i, :])

        def emit_q_head(n, m, xn):
            # one head's q projection + rope into q_sbs[n % 2]
            qp = pall.tile([P, 512], F32, tag="pa", name=f"qp{n}_{m}")
            for kc in range(KC):
                nc.tensor.matmul(qp[:], wq_sb[:, kc, bass.ts(m, P)],
                                 xn[:, kc, :],
                                 start=(kc == 0), stop=(kc == KC - 1))
            rope(q_sbs[n % 2][:, m, :], qp, n)

        # ---------- phase B per-head emitters ----------
        def emit_scores_chunk(n, h, js, spans, exp_sb):
            q_sb = q_sbs[n % 2]
            for j in js:
                g = spans[j]
                sl = slice(g, 512)
                scp = pall.tile([P, 512], F32, tag="pa",
                                name=f"scp{n}_{h}_{j}")
                nc.tensor.matmul(scp[:, sl], k_slabs[j // 4][:, j % 4, :],
                                 q_sb[:, h, sl], start=True, stop=True)
                nc.scalar.activation(exp_sb[:, j, sl], scp[:, sl],
                                     mybir.ActivationFunctionType.Exp,
                                     scale=SCALE)
                if j >= 4 * n:
                    # zero the in-block upper triangle (q < k) of the exp
                    tri = exp_sb[:, j, g:g + P]
                    nc.gpsimd.affine_select(
                        out=tri, in_=tri,
                        compare_op=mybir.AluOpType.is_ge,
                        fill=0.0, base=0,
                        pattern=[[1, P]], channel_multiplier=-1,
                    )

        def emit_esum_chunk(n, h, js, spans, exp_sb, esum):
            for j in js:
                sl = slice(spans[j], 512)
                if j == 0:
                    nc.vector.tensor_copy(esum[:], exp_sb[:, 0, :])
                else:
                    nc.vector.tensor_tensor(esum[:, sl], esum[:, sl],
                                            exp_sb[:, j, sl],
                                            mybir.AluOpType.add)

        def emit_zpv(n, h, nts, spans, exp_sb, esum):
            zp = pall.tile([P, 512], F32, tag="pa", name=f"zp{n}_{h}")
            nc.tensor.matmul(zp[:], ones[:], esum[:], start=True, stop=True)
            zr = bpool.tile([P, 512], F32, tag="zr")
            nc.vector.reciprocal_approx_fast(out=zr[:], in_=zp[:])

            yp = pall.tile([P, 512], F32, tag="pa", name=f"yp{n}_{h}")
            for j in range(nts):
                sl = slice(spans[j], 512)
                nc.tensor.matmul(yp[:, sl], v_slabs[j // 4][:, j % 4, :],
                                 exp_sb[:, j, sl],
                                 start=(j == 0), stop=(j == nts - 1))
            nc.vector.tensor_tensor(y_sbs[n % 2][:, h, :], yp[:], zr[:],
                                    mybir.AluOpType.mult)

        # ---------- phase C group emitter ----------
        def emit_c_group(n, t16, ci):
            y_sb = y_sbs[n % 2]
            ops = pall.tile([P, 512], F32, tag="pa", name=f"ops{n}_{t16}_{ci}")
            for h in range(HG):
                nc.tensor.matmul(ops[:], y_sb[:, h, bass.ts(t16, P)],
                                 wo_sb[:, h, bass.ts(ci, 512)],
                                 start=(h == 0), stop=(h == HG - 1))
            ost = rpool.tile([P, 512], BF16, tag="ost")
            nc.vector.tensor_copy(ost[:], ops[:])
            nc.sync.dma_start(
                outp[bass.ts(4 * n + t16, P), bass.ts(ci, 512)], ost[:])

        # =========================== schedule ===========================
        xns = [x0, None, None, None]

        def prefetch_x(n):
            xn = xpool.tile([P, KC, 512], BF16, tag="xn", name=f"xn{n}")
            nc.sync.dma_start(xn[:], xT_r[:, :, bass.ts(n, 512)])
            xns[n] = xn
            csns[n] = fetch_cs(n)

        # A(0)
        emit_kv(0, x0)
        for m in range(HG):
            emit_q_head(0, m, x0)

        for n in range(NQ):
            # fillers emitted inside B(n): C(n-1) groups then A(n+1) slices
            fillers = deque()
            if n > 0:
                for t16 in range(4):
                    for ci in range(4):
                        fillers.append(
                            lambda n=n, t16=t16, ci=ci: emit_c_group(
                                n - 1, t16, ci))
            if n + 1 < NQ:
                prefetch_x(n + 1)
                fillers.append(lambda n=n: emit_kv(n + 1, xns[n + 1]))
                for m in range(HG):
                    fillers.append(
                        lambda n=n, m=m: emit_q_head(n + 1, m, xns[n + 1]))

            nts = 4 * (n + 1)
            spans = [max(P * j - 512 * n, 0) for j in range(nts)]
            # budget fillers roughly evenly across the 8 head iterations;
            # software pipeline: z/PV of head h-1 are emitted after the
            # scores+exp of head h, so the PE has dense work while ACT
            # chews through head h's exps
            pending = None
            for h in range(HG):
                exp_sb = bpool.tile([P, NT, 512], BF16, tag="exp")
                esum = bpool.tile([P, 512], BF16, tag="esum")
                quota = (len(fillers) + (HG - h) - 1) // (HG - h)
                js = list(range(nts))
                # emit scores in chunks of 4 blocks, pulling a filler
                # between chunks so the PE never runs >4 psum banks ahead
                # of ACT and always has dense work
                ci = 0
                for c0 in range(0, nts, 4):
                    chunk = js[c0:c0 + 4]
                    emit_scores_chunk(n, h, chunk, spans, exp_sb)
                    emit_esum_chunk(n, h, chunk, spans, exp_sb, esum)
                    if ci < quota and fillers:
                        fillers.popleft()()
                        ci += 1
                if pending is not None:
                    emit_zpv(*pending)
                pending = (n, h, nts, spans, exp_sb, esum)
            emit_zpv(*pending)
            while fillers:
                fillers.popleft()()

        # C(3) tail
        for t16 in range(4):
            for ci in range(4):
                emit_c_group(NQ - 1, t16, ci)

    nc.finalize()
    return nc


_PERM = np.concatenate([np.arange(0, D, 2), np.arange(1, D, 2)])
_BF = ml_dtypes.bfloat16


def _prep_core_inputs(x, freqs_cos, freqs_sin, wq, wkv_down, wk_up, wv_up, wo):
    cosT = np.ascontiguousarray(freqs_cos.T).astype(np.float32)   # [64, T]
    sinT = np.ascontiguousarray(freqs_sin.T).astype(np.float32)
    c2 = np.concatenate([cosT, cosT], axis=0).astype(_BF)         # [128, T]
    s2 = np.concatenate([-sinT, sinT], axis=0).astype(_BF)

    wkvT = np.ascontiguousarray(wkv_down.T).astype(_BF)           # [C, L]
    wkuT = np.ascontiguousarray(wk_up[_PERM, :].T).astype(_BF)    # [L, D]
    wvuT = np.ascontiguousarray(wv_up.T).astype(_BF)              # [L, D]

    wq_h = wq.reshape(H, D, C)[:, _PERM, :]                       # perm rows/head

    in_maps = []
    for core in range(N_CORES):
        b, g = core // 2, core % 2
        heads = slice(8 * g, 8 * g + 8)
        wqT_g = np.ascontiguousarray(
            wq_h[heads].reshape(HG * D, C).T).astype(_BF)         # [C, 1024]
        woT_g = np.ascontiguousarray(
            wo[:, 8 * g * D:(8 * g + 8) * D].T).astype(_BF)       # [1024, C]
        xT_b = np.ascontiguousarray(x[b].T).astype(_BF)           # [C, T]
        in_maps.append({
            "xT": xT_b, "wqT": wqT_g, "wkvT": wkvT, "wkuT": wkuT,
            "wvuT": wvuT, "woT": woT_g, "c2": c2, "s2": s2,
        })
    return in_maps


def kernel(x, freqs_cos, freqs_sin, wq, wkv_down, wk_up, wv_up, wo, _trace=False):
    x = np.asarray(x, dtype=np.float32)
    freqs_cos = np.asarray(freqs_cos, dtype=np.float32)
    freqs_sin = np.asarray(freqs_sin, dtype=np.float32)
    wq = np.asarray(wq, dtype=np.float32)
    wkv_down = np.asarray(wkv_down, dtype=np.float32)
    wk_up = np.asarray(wk_up, dtype=np.float32)
    wv_up = np.asarray(wv_up, dtype=np.float32)
    wo = np.asarray(wo, dtype=np.float32)

    if "nc" not in _cached:
        _cached["nc"] = _build_program()
    nc = _cached["nc"]

    in_maps = _prep_core_inputs(x, freqs_cos, freqs_sin, wq, wkv_down,
                                wk_up, wv_up, wo)
    res = run_bass_kernel_spmd(nc, in_maps, core_ids=list(range(N_CORES)),
                               trace=_trace)
    _cached["last_result"] = res

    out = np.empty((B, T, C), dtype=np.float32)
    for b in range(B):
        out[b] = res.results[2 * b]["outp"] + res.results[2 * b + 1]["outp"]
    return out
